# revision 9
# baseline (speedup 1.0000x reference)
"""Cox partial-likelihood NegativeLogLikelihood loss on 8 Trainium2 cores.

reference:
    mask[i, j] = (y[j] <= y[i])                       # (N, N)
    num[j] = sum_i exp(r_i) * mask[i, j]
    den[j] = sum_i mask[i, j]
    loss = -sum_j e_j * (r_j - log(num_j / den_j)) / sum_j e_j + 0.01 * ||W||_F

Strategy: shard columns j across the 8 cores (each core owns 2048 columns).
The N x 2048 mask is materialized on-chip in [128, 2048] tiles and contracted
on the TensorEngine against lhsT = [exp_hi, exp_lo, 1] (bf16 Dekker split)
into PSUM.

Perf structure vs the first version:
  * y is re-encoded on the host as monotone bf16 codes (rank -> bf16 bit
    pattern + 0x2000), so comparisons are exact in bf16 and the DVE
    tensor_scalar(is_le) compare runs in the 4x perf mode (16-bit in/out,
    step 1) -- 2x faster mask production than the f32 compare.
  * Row-side code copies carry a +quarter-ulp offset so code_i' > code_j
    strictly for i == j and ties never occur: the ScalarE Sign producer
    yields exactly +/-1 and all tie/diagonal corrections vanish.
  * The matmuls are column-tiled 4 ways (tile_position=(0, 32g)): four
    thin-M (3-row) matmuls execute concurrently in disjoint 32-column
    strips of the PE array, quadrupling effective TensorE throughput for
    this N-bound shape.  PSUM group g accumulates i-tiles 32g..32g+31 at
    partitions 32g..32g+2.
  * The j-codes row is host-broadcast into a [128, 2048] bf16 input and
    loaded with one contiguous DMA (the on-the-fly broadcast DMA was
    ~30us in the old version).
Producers split 100 tiles on DVE (~594ns each) / 28 tiles on ACT (~2us
each, with halved weights and j-independent +V/2, +64 corrections).
Each core reduces to one scalar; the host unshard is an 8-way sum.
"""
import math

import numpy as np
import ml_dtypes
import orjson

import concourse.bass as bass
import concourse.tile as tile
import concourse.mybir as mybir
from concourse.bass_utils import run_bass_kernel_spmd

F32 = mybir.dt.float32
BF16 = mybir.dt.bfloat16

N = 16384
NCORES = 8
JSHARD = N // NCORES            # 2048 columns per core
NT = N // 128                   # 128 i-tiles of 128 rows
NG = 4                          # PE column-strip groups
NR = NT // NG                   # 32 i-tiles (rounds) per group
NJJ = JSHARD // 512             # 4 matmul column chunks per core
NACT = 28                       # ACT-produced tiles: group 3, rounds 0..27
C_HALF = float(NACT * 64)       # den correction: NACT tiles * 128 rows * 1/2


def tile_of(g, r):
    return 32 * g + r


def is_act(g, r):
    return g == 3 and r < NACT


ACT_TILES = [tile_of(3, r) for r in range(NACT)]

# ---------------------------------------------------------------------------
# Workaround for the installed walrus accepting at most ONE sync-wait command
# per TPB instruction: split multi-wait instructions into preceding
# single-wait EventSemaphore instructions on the same engine.
# ---------------------------------------------------------------------------


def _fix_bir_multiwait(bir_json: bytes) -> bytes:
    d = orjson.loads(bir_json)
    counter = 0
    for fn in d.get("functions", []):
        stack = list(fn.get("blocks", []))
        while stack:
            block = stack.pop()
            stack.extend(block.get("blocks", []))
            new_insts = []
            for inst in block.get("instructions", []):
                sync = inst.get("sync_info") or {}
                waits = sync.get("on_wait") or []
                if len(waits) > 1:
                    for w in waits[:-1]:
                        counter += 1
                        new_insts.append({
                            "debug": inst.get("debug", 0),
                            "engine": inst.get("engine"),
                            "ins": [],
                            "name": f"esw_fix_{counter}",
                            "opcode": "EventSemaphore",
                            "outs": [],
                            "sync_info": {"on_update": [], "on_wait": [w]},
                        })
                    sync["on_wait"] = [waits[-1]]
                new_insts.append(inst)
            block["instructions"] = new_insts
    return orjson.dumps(d)


_patched = False


def _install_bir_fix():
    global _patched
    if _patched:
        return
    _patched = True
    import concourse.bass_utils as bu
    import concourse.bass2jax as b2j

    orig = bu.compile_bir_kernel

    def patched(bir_json, tmpdir, neff_name="file.neff"):
        if isinstance(bir_json, str):
            bir_json = bir_json.encode()
        return orig(_fix_bir_multiwait(bir_json), tmpdir, neff_name)

    bu.compile_bir_kernel = patched
    b2j.compile_bir_kernel = patched


# ---------------------------------------------------------------------------
# Kernel build
# ---------------------------------------------------------------------------

def build_kernel() -> bass.Bass:
    nc = bass.Bass()
    Sign = mybir.ActivationFunctionType.Sign

    # j-side codes, host-broadcast to all 128 partitions, bf16
    yb_d = nc.dram_tensor("yb", [128, JSHARD], BF16, kind="ExternalInput")
    # column-major packs: [ycol(+delta) | rcol]
    colpack = nc.dram_tensor("colpack", [128, 2 * NT], F32, kind="ExternalInput")
    # misc: [r_pf | e_pf | e_f | w | scale_b | indh_b]
    MISC_W = 16 + 16 + NT + 1024 + NT + NT
    misc = nc.dram_tensor("misc", [128, MISC_W], F32, kind="ExternalInput")
    out = nc.dram_tensor("out", [1, 1], F32, kind="ExternalOutput")

    with tile.TileContext(nc) as tc:
        with (
            tc.tile_pool(name="const", bufs=1) as const,
            tc.tile_pool(name="masks", bufs=16) as masks,
            tc.tile_pool(name="psacc", bufs=1, space="PSUM") as psacc,
            tc.tile_pool(name="pswarm", bufs=1, space="PSUM") as pswarm,
            tc.tile_pool(name="pssum", bufs=1, space="PSUM") as pssum,
        ):
            # ---- critical-path loads
            yb = const.tile([128, JSHARD], BF16)
            nc.sync.dma_start(out=yb, in_=yb_d[:, :])
            col_sb = const.tile([128, 2 * NT], F32)
            nc.sync.dma_start(out=col_sb, in_=colpack[:, :])
            ycol_sb = col_sb[:, 0:NT]          # code_i + delta (f32)
            rcol_sb = col_sb[:, NT:2 * NT]     # risk_pred col-major (f32)
            misc_sb = const.tile([128, MISC_W], F32)
            nc.gpsimd.dma_start(out=misc_sb, in_=misc[:, :])
            o = 0
            rpf_sb = misc_sb[:, o:o + 16]; o += 16
            epf_f = misc_sb[:, o:o + 16]; o += 16
            e_f = misc_sb[:, o:o + NT]; o += NT
            w_sb = misc_sb[:, o:o + 1024]; o += 1024
            scale_b = misc_sb[:, o:o + NT]; o += NT    # 0.5 on ACT cols else 1
            indh_b = misc_sb[:, o:o + NT]; o += NT     # 0.5 on ACT cols else 0

            # ---- lhsT = scale * [exp_hi | exp_lo | ones | 0...] per i-tile.
            # Padded to 32 rows so each column-strip matmul writes its full
            # 32-partition PSUM block (rows 3..31 accumulate zeros); this
            # keeps every PSUM partition initialized for the wide epilogue
            # copy.  bf16.
            exp_sb = const.tile([128, NT], F32)
            nc.scalar.activation(exp_sb, rcol_sb, mybir.ActivationFunctionType.Exp)
            lhsT = const.tile([128, 32, NT], BF16)
            nc.gpsimd.memset(lhsT[:, :, :], 0.0)
            nc.vector.tensor_copy(lhsT[:, 0, :], exp_sb)          # hi = bf16(exp)
            hi32 = const.tile([128, NT], F32)
            nc.vector.tensor_copy(hi32, lhsT[:, 0, :])            # back to f32
            lo32 = const.tile([128, NT], F32)
            nc.vector.tensor_sub(lo32, exp_sb, hi32)              # f32 residual
            nc.vector.tensor_mul(lhsT[:, 0, :], hi32, scale_b)    # exact in bf16
            nc.vector.tensor_mul(lhsT[:, 1, :], lo32, scale_b)
            nc.vector.tensor_copy(lhsT[:, 2, :], scale_b)

            # ---- W frobenius^2 partial on ACT (early, accumulates to vec3)
            vec3 = const.tile([128, 3], F32)
            w2d = const.tile([128, 1024], F32)
            nc.scalar.activation(
                w2d, w_sb, mybir.ActivationFunctionType.Square,
                accum_out=vec3[:, 1:2],
            )

            # ---- V_half = sum(exp * indh) -> scalar -> broadcast to [128,1]
            vh = const.tile([128, NT], F32)
            nc.vector.tensor_mul(vh, exp_sb, indh_b)
            vred = const.tile([128, 1], F32)
            nc.vector.tensor_reduce(
                out=vred, in_=vh, axis=mybir.AxisListType.X, op=mybir.AluOpType.add)
            ones_col = const.tile([128, 1], F32)
            nc.vector.memset(ones_col, 1.0)
            va_ps = pssum.tile([1, 1], F32, name="va_ps")
            nc.tensor.matmul(va_ps[:, :], vred, ones_col, start=True, stop=True)
            va_row = const.tile([1, 1], F32)
            nc.vector.tensor_copy(va_row, va_ps[:, :])
            va_b = const.tile([128, 1], F32)
            va_dram = nc.dram_tensor("va_scratch", [1, 1], F32, kind="Internal")
            nc.gpsimd.dma_start(out=va_dram[:, :], in_=va_row)
            nc.gpsimd.dma_start(out=va_b, in_=va_dram.ap().to_broadcast([128, 1]))

            # ---- PE HAM warm-up: dummy matmuls so the real stream starts hot
            warm_ps = pswarm.tile([1, 256], F32)
            for k in range(4):
                nc.tensor.matmul(
                    warm_ps[:, :], ones_col, w_sb[:, 0:256],
                    start=True, stop=True, skip_group_check=True,
                )

            # ---- main loop: mask tiles + column-tiled matmul accumulation
            # acc group g lives at partitions [32g, 32g+3), banks by jj chunk
            acc = psacc.tile([128, NJJ * 512], F32)
            for r in range(NR):
                mt = {}
                for g in range(NG):
                    t = tile_of(g, r)
                    m = masks.tile([128, JSHARD], BF16)
                    mt[g] = m
                    if is_act(g, r):
                        nc.scalar.activation(
                            m, yb, Sign, bias=ycol_sb[:, t:t + 1], scale=-1.0,
                        )
                    else:
                        nc.vector.tensor_scalar(
                            out=m, in0=yb,
                            scalar1=ycol_sb[:, t:t + 1], scalar2=None,
                            op0=mybir.AluOpType.is_le,
                        )
                for jj in range(NJJ):
                    for g in range(NG):
                        t = tile_of(g, r)
                        nc.tensor.matmul(
                            acc[32 * g:32 * g + 32, 512 * jj:512 * (jj + 1)],
                            lhsT[:, :, t],
                            mt[g][:, 512 * jj:512 * (jj + 1)],
                            start=(r == 0), stop=(r == NR - 1),
                            tile_position=(0, 32 * g),
                        )

            # ---- e_sum partial (overlaps main loop)
            nc.vector.tensor_reduce(
                out=vec3[:, 0:1], in_=e_f, axis=mybir.AxisListType.X,
                op=mybir.AluOpType.add,
            )

            # ---- epilogue: PSUM -> SBUF staging (split across both PSUM-
            # capable engines), then scatter rows into pf layout [128, 16, NG]
            # pf mapping: x_pf[p, c] = x_shard[16*p + c]
            nd_all = const.tile([128, NJJ * 512], F32)
            nc.scalar.copy(nd_all[:, 0:1024], acc[:, 0:1024])
            nc.vector.tensor_copy(nd_all[:, 1024:2048], acc[:, 1024:2048])
            hi_pf = const.tile([128, 16, NG], F32)
            lo_pf = const.tile([128, 16, NG], F32)
            den_pf = const.tile([128, 16, NG], F32)
            dmaeng = [nc.sync, nc.gpsimd, nc.scalar]
            for g in range(NG):
                for k, dst in enumerate((hi_pf, lo_pf, den_pf)):
                    dmaeng[(3 * g + k) % 3].dma_start(
                        out=dst[:, :, g],
                        in_=nd_all[32 * g + k:32 * g + k + 1, :])

            hi_s = const.tile([128, 16, 1], F32)
            lo_s = const.tile([128, 16, 1], F32)
            den_s = const.tile([128, 16, 1], F32)
            nc.vector.tensor_reduce(
                out=hi_s, in_=hi_pf, axis=mybir.AxisListType.X,
                op=mybir.AluOpType.add)
            nc.vector.tensor_reduce(
                out=lo_s, in_=lo_pf, axis=mybir.AxisListType.X,
                op=mybir.AluOpType.add)
            nc.vector.tensor_reduce(
                out=den_s, in_=den_pf, axis=mybir.AxisListType.X,
                op=mybir.AluOpType.add)
            hi2 = hi_s[:, :, 0]
            lo2 = lo_s[:, :, 0]
            den2 = den_s[:, :, 0]

            # ---- wide final math on [128, 16]
            n1 = const.tile([128, 16], F32)
            nc.vector.tensor_add(n1, hi2, lo2)
            num_pf = const.tile([128, 16], F32)
            nc.vector.tensor_scalar(
                out=num_pf, in0=n1, scalar1=va_b[:, 0:1], scalar2=None,
                op0=mybir.AluOpType.add)                          # + V_half
            den_c = const.tile([128, 16], F32)
            nc.vector.tensor_scalar(
                out=den_c, in0=den2, scalar1=C_HALF, scalar2=None,
                op0=mybir.AluOpType.add)                          # + C_half
            lnn = const.tile([128, 16], F32)
            nc.scalar.activation(lnn, num_pf, mybir.ActivationFunctionType.Ln)
            lnd = const.tile([128, 16], F32)
            nc.scalar.activation(lnd, den_c, mybir.ActivationFunctionType.Ln)
            s1 = const.tile([128, 16], F32)
            nc.vector.tensor_sub(s1, rpf_sb, lnn)
            s2 = const.tile([128, 16], F32)
            nc.vector.tensor_add(s2, s1, lnd)
            s3 = const.tile([128, 16], F32)
            nc.vector.tensor_mul(s3, s2, epf_f)
            nc.vector.tensor_reduce(
                out=vec3[:, 2:3], in_=s3, axis=mybir.AxisListType.X,
                op=mybir.AluOpType.add,
            )

            # ---- cross-partition fold: [e_sum, w_ssq, t_sum] into one row
            sums = pssum.tile([1, 3], F32)
            nc.tensor.matmul(sums[:, :], ones_col, vec3[:, :], start=True, stop=True)

            # ---- assemble out_c = -t_sum / e_sum + (0.01/8) * sqrt(w_ssq)
            inv_e = const.tile([1, 1], F32)
            nc.vector.reciprocal(inv_e, sums[0:1, 0:1])
            lnw = const.tile([1, 1], F32)
            nc.scalar.activation(lnw, sums[0:1, 1:2], mybir.ActivationFunctionType.Ln)
            f1 = const.tile([1, 1], F32)
            # 0.00125 * sqrt(w_ssq) = exp(0.5 * ln(w_ssq) + ln(0.00125))
            lbias = const.tile([1, 1], F32)
            nc.vector.memset(lbias, math.log(0.01 / NCORES))
            nc.scalar.activation(
                f1, lnw, mybir.ActivationFunctionType.Exp,
                scale=0.5, bias=lbias,
            )
            tsc = const.tile([1, 1], F32)
            nc.vector.tensor_mul(tsc, sums[0:1, 2:3], inv_e)
            res = const.tile([1, 1], F32)
            nc.vector.tensor_sub(res, f1, tsc)
            nc.gpsimd.dma_start(out=out[:, :], in_=res)

    return nc


_nc_cache = None


def _get_nc():
    global _nc_cache
    if _nc_cache is None:
        _install_bir_fix()
        _nc_cache = build_kernel()
    return _nc_cache


def make_in_maps(risk_pred, y, e, W):
    """Host-side sharding: slice/reshape/encode the full inputs per core."""
    yflat = y.reshape(-1)
    # monotone distinct bf16 codes: rank -> bf16 bit pattern (+0x2000 keeps
    # every code and its successor a normal number in [2^-63, 2^64], so
    # all pairwise differences are far from f32 under/overflow)
    order = np.argsort(yflat, kind="stable")
    ranks = np.empty(N, np.uint16)
    ranks[order] = np.arange(N, dtype=np.uint16)
    codes_u16 = (ranks + np.uint16(0x2000)).astype(np.uint16)
    codes_bf16 = codes_u16.view(ml_dtypes.bfloat16)
    codes_f32 = codes_bf16.astype(np.float32)
    nxt_f32 = (codes_u16 + np.uint16(1)).view(ml_dtypes.bfloat16).astype(np.float32)
    # row-side codes get +quarter-gap so the diagonal compare is strict (+1)
    ycol_delta = codes_f32 + 0.25 * (nxt_f32 - codes_f32)

    ycol = ycol_delta.reshape(NT, 128).T                     # [p, t]
    rcol = risk_pred.reshape(NT, 128).T.astype(np.float32)
    ef = e.astype(np.float32).reshape(NT, 128).T
    colpack = np.ascontiguousarray(
        np.concatenate([ycol, rcol], axis=1), dtype=np.float32)
    w_flat = W.reshape(128, 1024).astype(np.float32)
    act_mask = np.zeros(NT, np.float32)
    act_mask[ACT_TILES] = 1.0
    scale_b = np.tile(1.0 - 0.5 * act_mask, (128, 1)).astype(np.float32)
    indh_b = np.tile(0.5 * act_mask, (128, 1)).astype(np.float32)

    in_maps = []
    for c in range(NCORES):
        j0 = c * JSHARD
        rsh = risk_pred.reshape(-1)[j0:j0 + JSHARD]
        esh = e.astype(np.float32).reshape(-1)[j0:j0 + JSHARD]
        r_pf = rsh.reshape(128, 16).astype(np.float32)
        e_pf = esh.reshape(128, 16)
        misc = np.ascontiguousarray(np.concatenate(
            [r_pf, e_pf, ef, w_flat, scale_b, indh_b], axis=1),
            dtype=np.float32)
        yb = np.ascontiguousarray(
            np.broadcast_to(codes_bf16[j0:j0 + JSHARD], (128, JSHARD)))
        in_maps.append(dict(yb=yb, colpack=colpack, misc=misc))
    return in_maps


def kernel(risk_pred, y, e, W, **run_kwargs):
    nc = _get_nc()
    in_maps = make_in_maps(
        np.asarray(risk_pred, np.float32),
        np.asarray(y, np.float32),
        np.asarray(e, np.int32),
        np.asarray(W, np.float32),
    )
    result = run_bass_kernel_spmd(nc, in_maps, core_ids=list(range(NCORES)),
                                  **run_kwargs)
    total = np.float32(0.0)
    for r in result.results:
        total = np.float32(total + r["out"][0, 0])
    kernel.last_result = result
    return np.asarray(total, np.float32)
